# revision 5
# baseline (speedup 1.0000x reference)
"""Trainium2 Bass kernel for nn_ClaimEncoder (dense_mlp).

Math (per row):
  feats = [sin/cos point-encoders (2x256), leaky number-encoders (3x128)]  -> [896]
  h   = leaky_relu(feats @ W1 + b1)   -> [512]
  out = leaky_relu(h @ W2 + b2)       -> [512]

Strategy: pure data parallel over 8 NeuronCores (16384 rows each).

Device-side design (per core, batch tiles of NB=512 columns):
  * The whole encoder is an affine map: with A8 = [x_s, y_s, x_d, y_d, t,
    ws, wd, 1] (8 values per row), each of the 7 128-feature chunks of
    featsT is  act(P_chunk.T @ A8_tile)  -- one K=8 matmul per chunk.
    cos(z) is computed as sin(z + pi/2) by baking pi/2 into the bias row
    of P; ACT's Sin is only valid on [-pi, pi] so the bias row also adds
    +pi+32pi and the DVE applies (z mod 2pi) - pi on PSUM eviction.
  * featsT comes out feature-major [feat, batch] - exactly the K-layout
    the L1 matmul needs for both operands.  L1: hT = W1_chunk.T @ featsT
    (bias b1 + leaky fused into the ACT eviction, b1 is per-partition).
  * L2 uses hT as the *stationary* operand (lhsT = hT chunk, rhs = W2
    chunk) which lands the output batch-major in PSUM -> contiguous DMA
    to DRAM.  b2 varies along the free dim so DVE adds broadcast b2
    during eviction, ACT applies the leaky relu.
  * All matmuls use float32r (1 cycle/row at N=512 vs 4 for plain fp32).
"""

import numpy as np

import concourse.bass as bass
import concourse.tile as tile
import concourse.mybir as mybir
from concourse import bacc
from concourse.bass_utils import run_bass_kernel_spmd

# Problem shapes (hardcoded; kernel.py must be self-contained).
B = 131072
N_CORES = 8
BC = B // N_CORES          # 16384 rows per core
PED = 256
NED = 128
CED = 512
Q = PED // 4               # 64
FEAT = 2 * PED + 3 * NED   # 896
NB = 512                   # batch columns per matmul tile
N_TILES = BC // NB         # 32
KC = FEAT // 128           # 7 feature chunks
MC = CED // 128            # 4 output chunks

TWO_PI = 2.0 * np.pi
# fp32 round-to-nearest-integer magic constant: adding it forces the
# mantissa to integer granularity (valid for |x| << 2^22).
MAGIC = 1.5 * 2.0 ** 23

F32 = mybir.dt.float32
F32R = mybir.dt.float32r


def _r(ap):
    """Bitcast an f32 AP to float32r for full-rate PE matmul."""
    return ap.bitcast(F32R)


def _build_bass():
    nc = bacc.Bacc(
        "TRN2",
        target_bir_lowering=False,
        debug=False,
        enable_asserts=False,
        num_devices=N_CORES,
    )

    a8 = nc.dram_tensor("a8", [8, BC], F32R, kind="ExternalInput").ap()
    p = nc.dram_tensor("p", [8, FEAT], F32R, kind="ExternalInput").ap()
    w1 = nc.dram_tensor("w1", [FEAT, CED], F32R, kind="ExternalInput").ap()
    w2 = nc.dram_tensor("w2", [CED, CED], F32R, kind="ExternalInput").ap()
    b1 = nc.dram_tensor("b1", [CED], F32, kind="ExternalInput").ap()
    b2 = nc.dram_tensor("b2", [CED], F32, kind="ExternalInput").ap()
    out = nc.dram_tensor("out", [BC, CED], F32, kind="ExternalOutput").ap()

    with tile.TileContext(nc) as tc:
        with (
            tc.tile_pool(name="consts", bufs=1) as consts,
            tc.tile_pool(name="a8p", bufs=4) as a8_pool,
            tc.tile_pool(name="featsp", bufs=2) as feats_pool,
            tc.tile_pool(name="hp", bufs=2) as h_pool,
            tc.tile_pool(name="rrp", bufs=4) as rr_pool,
            tc.tile_pool(name="l2tmp", bufs=4) as l2tmp_pool,
            tc.tile_pool(name="outp", bufs=6) as out_pool,
            tc.tile_pool(name="enc_ps", bufs=2, space="PSUM") as enc_psum,
            tc.tile_pool(name="l1_ps", bufs=3, space="PSUM") as l1_psum,
            tc.tile_pool(name="l2_ps", bufs=3, space="PSUM") as l2_psum,
        ):
            # ---- one-time constant loads ----
            p_sb = consts.tile([8, FEAT], F32R)
            nc.sync.dma_start(out=p_sb[:], in_=p[:, :])

            # W1 stored k-chunk-major: block c (cols c*512..) = W1[c*128:(c+1)*128, :]
            w1_sb = consts.tile([128, KC * CED], F32R)
            for c in range(KC):
                nc.sync.dma_start(
                    out=w1_sb[:, c * CED:(c + 1) * CED],
                    in_=w1[c * 128:(c + 1) * 128, :],
                )
            w2_sb = consts.tile([128, MC * CED], F32R)
            for k in range(MC):
                nc.sync.dma_start(
                    out=w2_sb[:, k * CED:(k + 1) * CED],
                    in_=w2[k * 128:(k + 1) * 128, :],
                )
            # b1 per-chunk columns: b1_sb[p, m] = b1[m*128 + p]
            b1_sb = consts.tile([128, MC], F32)
            nc.sync.dma_start(out=b1_sb[:], in_=b1.rearrange("(m q) -> q m", q=128))
            # b2 broadcast across partitions: b2b[p, f] = b2[f]
            b2b_sb = consts.tile([128, CED], F32)
            b2_bcast = bass.AP(
                tensor=b2.tensor, offset=b2.offset, ap=[[0, 128]] + list(b2.ap)
            )
            nc.gpsimd.dma_start(out=b2b_sb[:], in_=b2_bcast)

            for t in range(N_TILES):
                bt = t * NB
                a8_t = a8_pool.tile([8, NB], F32R)
                nc.sync.dma_start(out=a8_t[:], in_=a8[:, bt:bt + NB])

                # ---- encoder: featsT chunks [128 feat, NB batch] ----
                feats = feats_pool.tile([128, KC * NB], F32R)
                for c in range(KC):
                    eps_t = enc_psum.tile([128, NB], F32, name=f"eps_{t}_{c}", tag="eps")
                    nc.tensor.matmul(
                        eps_t[:],
                        p_sb[:, c * 128:(c + 1) * 128],
                        a8_t[:],
                        start=True,
                        stop=True,
                    )
                    dst = feats[:, c * NB:(c + 1) * NB]
                    if c < 4:
                        # PSUM holds z' = z/2pi (P pre-scaled on host).
                        # Range-reduce: k = round(z'), y = k - z' in
                        # [-0.5, 0.5], then sin(-2pi*y) = sin(z).
                        rr = rr_pool.tile([128, NB], F32, name=f"rr_{t}_{c}", tag="rr")
                        nc.vector.tensor_scalar_add(rr[:], eps_t[:], MAGIC)
                        rr2 = rr_pool.tile([128, NB], F32, name=f"rr2_{t}_{c}", tag="rr2")
                        nc.vector.scalar_tensor_tensor(
                            rr2[:], rr[:], MAGIC, eps_t[:],
                            op0=mybir.AluOpType.subtract,
                            op1=mybir.AluOpType.subtract,
                        )
                        nc.scalar.activation(
                            dst, rr2[:], mybir.ActivationFunctionType.Sin,
                            scale=-TWO_PI,
                        )
                    else:
                        nc.scalar.activation(
                            dst, eps_t[:], mybir.ActivationFunctionType.Prelu,
                            alpha=0.01,
                        )

                # ---- L1: hT chunks [128 feat_out, NB batch] ----
                h = h_pool.tile([128, MC * NB], F32R)
                for m in range(MC):
                    l1p = l1_psum.tile([128, NB], F32, name=f"l1p_{t}_{m}", tag="l1p")
                    for c in range(KC):
                        nc.tensor.matmul(
                            l1p[:],
                            w1_sb[:, c * CED + m * 128: c * CED + (m + 1) * 128],
                            feats[:, c * NB:(c + 1) * NB],
                            start=(c == 0),
                            stop=(c == KC - 1),
                        )
                    nc.scalar.activation(
                        h[:, m * NB:(m + 1) * NB], l1p[:],
                        mybir.ActivationFunctionType.Prelu,
                        bias=b1_sb[:, m:m + 1], alpha=0.01,
                    )

                # ---- L2: batch-major out chunks [128 batch, 512 feat] ----
                for j in range(MC):
                    l2p = l2_psum.tile([128, NB], F32, name=f"l2p_{t}_{j}", tag="l2p")
                    for k in range(MC):
                        nc.tensor.matmul(
                            l2p[:],
                            h[:, k * NB + j * 128: k * NB + (j + 1) * 128],
                            w2_sb[:, k * CED:(k + 1) * CED],
                            start=(k == 0),
                            stop=(k == MC - 1),
                        )
                    l2t = l2tmp_pool.tile([128, NB], F32, name=f"l2t_{t}_{j}", tag="l2t")
                    nc.vector.tensor_tensor(
                        l2t[:], l2p[:], b2b_sb[:], op=mybir.AluOpType.add
                    )
                    osb = out_pool.tile([128, NB], F32, name=f"osb_{t}_{j}", tag="osb")
                    nc.scalar.activation(
                        osb[:], l2t[:], mybir.ActivationFunctionType.Prelu, alpha=0.01
                    )
                    nc.sync.dma_start(
                        out=out[bt + j * 128: bt + (j + 1) * 128, :], in_=osb[:]
                    )

    nc.compile()
    return nc


def _host_pack(inputs):
    """Build A8 [8, B] and the encoder matrix P [8, FEAT] (bias folded)."""
    f32 = lambda k: np.ascontiguousarray(np.asarray(inputs[k], dtype=np.float32))
    src = f32("src_xy")
    dst = f32("dst_xy")

    a8 = np.empty((8, B), np.float32)
    a8[0] = src[:, 0]
    a8[1] = src[:, 1]
    a8[2] = dst[:, 0]
    a8[3] = dst[:, 1]
    a8[4] = f32("time_s")
    a8[5] = f32("wait_src")
    a8[6] = f32("wait_dst")
    a8[7] = 1.0

    p = np.zeros((8, FEAT), np.float32)
    for e, pfx in enumerate(("src", "dst")):
        base = e * PED
        vrow = {0: 2 * e, 1: 2 * e + 1}  # x row, y row in A8
        for bi, (wname, bname, axis, phase) in enumerate((
            ("wsx", "bsx", 0, 0.0),
            ("wcx", "bcx", 0, np.pi / 2),
            ("wsy", "bsy", 1, 0.0),
            ("wcy", "bcy", 1, np.pi / 2),
        )):
            # Point blocks are pre-scaled by 1/2pi: PSUM gets z' = z/2pi,
            # the DVE range-reduces, ACT applies sin(-2pi * (k - z')).
            cols = slice(base + bi * Q, base + (bi + 1) * Q)
            p[vrow[axis], cols] = f32(f"{pfx}_{wname}") / TWO_PI
            p[7, cols] = (f32(f"{pfx}_{bname}") + phase) / TWO_PI
    for i, pfx in enumerate(("t", "ws", "wd")):
        cols = slice(2 * PED + i * NED, 2 * PED + (i + 1) * NED)
        p[4 + i, cols] = f32(f"{pfx}_w")
        p[7, cols] = f32(f"{pfx}_b")

    w1 = f32("W1")
    b1 = f32("b1")
    w2 = f32("W2")
    b2 = f32("b2")
    return a8, p, w1, b1, w2, b2


_NC_CACHE = []


def kernel(**inputs) -> np.ndarray:
    a8, p, w1, b1, w2, b2 = _host_pack(inputs)

    if not _NC_CACHE:
        _NC_CACHE.append(_build_bass())
    nc = _NC_CACHE[0]

    in_maps = []
    for i in range(N_CORES):
        in_maps.append({
            "a8": np.ascontiguousarray(a8[:, i * BC:(i + 1) * BC]),
            "p": p,
            "w1": w1,
            "w2": w2,
            "b1": b1,
            "b2": b2,
        })

    res = run_bass_kernel_spmd(nc, in_maps, core_ids=list(range(N_CORES)))
    return np.concatenate([r["out"] for r in res.results], axis=0)
